# revision 4
# baseline (speedup 1.0000x reference)
"""Causal self-attention on 8 NeuronCores (Bass/Tile).

Sharding: tensor-parallel over heads x data-parallel over batch.
  core c -> batch b = c//4, heads 4g..4g+3 where g = c%4.
Each core computes q,k,v for its 4 heads (over its batch's 2048 tokens),
causal softmax attention, and the partial output projection over its 256
head-channels. Host sums the 4 partials per batch and adds b_proj.

v2 design (cost model: matmul time = out_free_size x cyc/row, rhs dtype
keyed; bf16 = 1 cyc/row at any width):
- QKV projection in f32r (full rate at free>=256), biases folded in via
  DVE adds that double as the psum->sbuf drain; q/k land in bf16.
- Scores S[kt,q] in bf16 (64-deep contraction, psum [128, 2 heads, 512]),
  exp fused across 2 heads per ACT instruction (halves ACT instr count).
- p@v computed TRANSPOSED: py[q, d+1] = p[kt,q]^T @ vaug[kt, d|1], so the
  matmul free dim is 65 instead of the 128..512 query width -- pv PE rows
  drop 2x. Denominator rides along as vaug's ones column; the normalize
  becomes a per-partition-scalar broadcast multiply (no partition
  broadcast needed). y is then transposed back with PE transpose-mode
  matmuls (128 rows each) for the output projection.
- Output projection in bf16 from the transposed yT, drained by DVE,
  written f32 to DRAM.
The per-512-token stripes are emitted interleaved; proj for q-tile tg is
emitted one tile behind (lag-1) so its PE work fills the ACT-bound tail.
"""

import os
import sys

for _p in ("/opt/trn_rl_repo", "/opt/pypackages"):
    if os.path.isdir(_p) and _p not in sys.path:
        sys.path.append(_p)

import numpy as np

import concourse.bass as bass
import concourse.tile as tile
import concourse.mybir as mybir
from concourse import bacc
from concourse.bass_utils import run_bass_kernel_spmd

B, T, C = 2, 2048, 1024
H = 16            # total heads
D = 64            # head dim
HPC = 4           # heads per core
CH = HPC * D      # 256 channels per core
N_CORES = 8

f32 = mybir.dt.float32
f32r = mybir.dt.float32r
bf16 = mybir.dt.bfloat16
ts = bass.ts
ds = bass.ds
AF = mybir.ActivationFunctionType

_COMPILED = None


def _build():
    nc = bacc.Bacc("TRN2", target_bir_lowering=False, debug=False,
                   num_devices=N_CORES)

    xT = nc.dram_tensor("xT", [C, T], f32, kind="ExternalInput").ap()
    wt = nc.dram_tensor("wt", [C, 3 * CH], f32, kind="ExternalInput").ap()
    wpt = nc.dram_tensor("wpt", [CH, C], f32, kind="ExternalInput").ap()
    bqk = nc.dram_tensor("bqk", [128, 4], f32, kind="ExternalInput").ap()
    bvb = nc.dram_tensor("bvb", [128, HPC, D], f32, kind="ExternalInput").ap()
    Sm = nc.dram_tensor("Sm", [128, 128], f32, kind="ExternalInput").ap()
    Idm = nc.dram_tensor("Idm", [128, 128], f32, kind="ExternalInput").ap()
    out = nc.dram_tensor("out_partial", [T, C], f32, kind="ExternalOutput").ap()

    NT512 = T // 512          # 4   512-token stripes
    NT128 = T // 128          # 16  128-token tiles
    NC128 = C // 128          # 8   contraction tiles

    with tile.TileContext(nc) as tc:
        with tc.tile_pool(name="consts", bufs=1) as consts, \
             tc.tile_pool(name="qkv", bufs=1) as qkv, \
             tc.tile_pool(name="xp", bufs=2) as xp, \
             tc.tile_pool(name="pp", bufs=17) as pp, \
             tc.tile_pool(name="yn", bufs=2) as yn, \
             tc.tile_pool(name="op", bufs=3) as op, \
             tc.tile_pool(name="ps_s", bufs=2, space="PSUM") as ps_s, \
             tc.tile_pool(name="ps_y", bufs=2, space="PSUM") as ps_y, \
             tc.tile_pool(name="ps_big", bufs=2, space="PSUM") as ps_big:

            # ---- constants; DMA emission order puts stripe-0 essentials
            #      (xt0 chunks + qk half of wt) first ----
            xT_r = xT.rearrange("(o p) t -> p o t", p=128).bitcast(f32r)
            wt_r = wt.rearrange("(o p) f -> p o f", p=128).bitcast(f32r)
            wt_sb = consts.tile([128, NC128, 3 * CH], f32r)
            xt0 = xp.tile([128, NC128, 512], f32r, tag="xt")
            for ci in range(NC128):
                nc.sync.dma_start(xt0[:, ci], xT_r[:, ci, ts(0, 512)])
                nc.sync.dma_start(wt_sb[:, ci, :512], wt_r[:, ci, :512])
            bqk_sb = consts.tile([128, 4], f32)
            nc.sync.dma_start(bqk_sb[:], bqk)
            nc.sync.dma_start(wt_sb[:, :, 512:], wt_r[:, :, 512:])
            bvb_sb = consts.tile([128, HPC, D], f32)
            nc.sync.dma_start(bvb_sb[:], bvb)
            S_f = consts.tile([128, 128], f32)
            nc.sync.dma_start(S_f[:], Sm)
            Id_f = consts.tile([128, 128], f32)
            nc.sync.dma_start(Id_f[:], Idm)
            wpt_f = consts.tile([128, 2, C], f32)
            nc.sync.dma_start(
                wpt_f[:], wpt.rearrange("(s p) o -> p s o", p=128))

            S_sb = consts.tile([128, 128], bf16)
            nc.vector.tensor_copy(S_sb[:], S_f[:])
            Id_sb = consts.tile([128, 128], bf16)
            nc.vector.tensor_copy(Id_sb[:], Id_f[:])
            wpt_sb = consts.tile([128, 2, C], bf16)
            nc.vector.tensor_copy(wpt_sb[:], wpt_f[:])

            # ---- persistent activations ----
            qT = qkv.tile([128, 2, T], bf16)      # [2h*64, slab, t]
            kT = qkv.tile([128, 2, T], bf16)
            vaug = qkv.tile([128, NT128, HPC, D + 1], bf16)  # [kt, ki, h, d|1]
            yT = qkv.tile([128, 2, T], bf16)

            nc.vector.memset(vaug[:, :, :, D:D + 1], 1.0)

            def emit_proj(tg):
                for oi in range(2):
                    po = ps_big.tile([128, 512], f32, tag="big")
                    for s in range(2):
                        nc.tensor.matmul(
                            po[:], yT[:, s, ts(tg, 128)],
                            wpt_sb[:, s, ts(oi, 512)],
                            start=(s == 0), stop=(s == 1))
                    ot = op.tile([128, 512], f32, tag="ot")
                    nc.vector.tensor_copy(ot[:], po[:])
                    nc.sync.dma_start(out[ts(tg, 128), ts(oi, 512)], ot[:])

            for ti in range(NT512):
                # ---------- QKV projection for stripe ti ----------
                if ti == 0:
                    xt = xt0
                else:
                    xt = xp.tile([128, NC128, 512], f32r, tag="xt")
                    nc.sync.dma_start(xt[:], xT_r[:, :, ts(ti, 512)])
                for fj in range(4):          # q0 q1 k0 k1
                    ps = ps_big.tile([128, 512], f32, tag="big")
                    for ci in range(NC128):
                        nc.tensor.matmul(
                            ps[:], wt_sb[:, ci, ts(fj, 128)], xt[:, ci, :],
                            start=(ci == 0), stop=(ci == NC128 - 1))
                    dest = qT if fj < 2 else kT
                    nc.vector.tensor_add(
                        out=dest[:, fj % 2, ts(ti, 512)], in0=ps[:],
                        in1=bqk_sb[:, fj:fj + 1].to_broadcast([128, 512]))
                for tj in range(4):
                    pv = ps_big.tile([128, HPC, D], f32, tag="big")
                    for ci in range(NC128):
                        nc.tensor.matmul(
                            pv[:, :, :], xt[:, ci, ts(tj, 128)],
                            wt_sb[:, ci, 512:512 + CH],
                            start=(ci == 0), stop=(ci == NC128 - 1))
                    nc.vector.tensor_add(
                        out=vaug[:, 4 * ti + tj, :, 0:D],
                        in0=pv[:, :, :], in1=bvb_sb[:])

                # ---------- attention stripe qi = ti ----------
                qi = ti
                nk = 4 * qi + 4
                p4s = []
                for ki in range(nk):
                    j = ki - 4 * qi
                    q0 = max(0, 128 * j)
                    w = 512 - q0
                    p4 = pp.tile([128, HPC, 512], bf16, tag="p4")
                    for g in range(2):
                        sc = ps_s.tile([128, 2, 512], f32, tag="sc")
                        for hh in range(2):
                            nc.tensor.matmul(
                                sc[:, hh, q0:],
                                kT[ts(hh, D), g, ts(ki, 128)],
                                qT[ts(hh, D), g, ds(512 * qi + q0, w)],
                                start=True, stop=True)
                        nc.scalar.activation(
                            p4[:, ts(g, 2), q0:], sc[:, :, q0:], AF.Exp)
                        if j >= 0:
                            for hh in range(2):
                                nc.vector.tensor_mul(
                                    out=p4[:, 2 * g + hh, q0:q0 + 128],
                                    in0=p4[:, 2 * g + hh, q0:q0 + 128],
                                    in1=S_sb[:])
                    p4s.append(p4)

                # ---------- pv (transposed) + normalize + transpose ----------
                for tg_rel in range(4):
                    tg = 4 * qi + tg_rel
                    py4 = ps_y.tile([128, HPC, D + 1], f32, tag="py")
                    for h in range(HPC):
                        for ki in range(tg + 1):
                            nc.tensor.matmul(
                                py4[:, h, :],
                                p4s[ki][:, h, ts(tg_rel, 128)],
                                vaug[:, ki, h, :],
                                start=(ki == 0), stop=(ki == tg))
                    rec4 = yn.tile([128, HPC, 1], f32, tag="rec")
                    nc.vector.reciprocal(rec4[:], py4[:, :, D:D + 1])
                    y_n = yn.tile([128, HPC, D], bf16, tag="yn")
                    nc.vector.tensor_mul(
                        out=y_n[:], in0=py4[:, :, 0:D],
                        in1=rec4.to_broadcast([128, HPC, D]))
                    yTt = ps_y.tile([128, 2, 128], bf16, tag="py")
                    for i in range(2):
                        nc.tensor.transpose(
                            yTt[:, i, :], y_n[:, ts(i, 2), :], Id_sb[:])
                    nc.vector.tensor_copy(yT[:, :, ts(tg, 128)], yTt[:])
                    if tg >= 1:
                        emit_proj(tg - 1)

            emit_proj(NT128 - 1)

    nc.compile()
    return nc


def _get_compiled():
    global _COMPILED
    if _COMPILED is None:
        _COMPILED = _build()
    return _COMPILED


def _host_prep(x, W_attn, b_attn, W_proj, b_proj):
    scale = 1.0 / np.sqrt(np.float32(D))
    xTb = [np.ascontiguousarray(x[b].T).astype(np.float32) for b in range(B)]
    Sm = (np.arange(128, dtype=np.int32)[None, :]
          >= np.arange(128, dtype=np.int32)[:, None]).astype(np.float32)
    Idm = np.eye(128, dtype=np.float32)
    in_maps = []
    for c in range(N_CORES):
        b, g = divmod(c, 4)
        ch = slice(CH * g, CH * (g + 1))
        Wq = W_attn[ch]
        Wk = W_attn[C:][ch] * scale
        Wv = W_attn[2 * C:][ch]
        wt_c = np.ascontiguousarray(
            np.concatenate([Wq, Wk, Wv], axis=0).T).astype(np.float32)
        bq = b_attn[ch]
        bk = b_attn[C:][ch] * scale
        bv = b_attn[2 * C:][ch]
        bqk_c = np.ascontiguousarray(
            np.concatenate([bq, bk]).reshape(4, 128).T).astype(np.float32)
        bvb_c = np.ascontiguousarray(
            np.broadcast_to(bv[None, :].reshape(1, HPC, D),
                            (128, HPC, D))).astype(np.float32)
        wpt_c = np.ascontiguousarray(W_proj[:, ch].T).astype(np.float32)
        in_maps.append({
            "xT": xTb[b],
            "wt": wt_c,
            "wpt": wpt_c,
            "bqk": bqk_c,
            "bvb": bvb_c,
            "Sm": Sm,
            "Idm": Idm,
        })
    return in_maps


def kernel(x, W_attn, b_attn, W_proj, b_proj):
    x = np.asarray(x, dtype=np.float32)
    W_attn = np.asarray(W_attn, dtype=np.float32)
    b_attn = np.asarray(b_attn, dtype=np.float32)
    W_proj = np.asarray(W_proj, dtype=np.float32)
    b_proj = np.asarray(b_proj, dtype=np.float32)

    nc = _get_compiled()
    in_maps = _host_prep(x, W_attn, b_attn, W_proj, b_proj)
    res = run_bass_kernel_spmd(nc, in_maps, core_ids=list(range(N_CORES)))

    out = np.empty((B, T, C), dtype=np.float32)
    for b in range(B):
        acc = res.results[4 * b]["out_partial"].copy()
        for g in range(1, 4):
            acc += res.results[4 * b + g]["out_partial"]
        out[b] = acc + b_proj
    return out


# revision 5
# speedup vs baseline: 1.2050x; 1.2050x over previous
"""Causal self-attention on 8 NeuronCores (Bass/Tile).

Sharding: tensor-parallel over heads x data-parallel over batch.
  core c -> batch b = c//4, heads 4g..4g+3 where g = c%4.
Each core computes q,k,v for its 4 heads (over its batch's 2048 tokens),
causal softmax attention, and the partial output projection over its 256
head-channels. Host sums the 4 partials per batch and adds b_proj.

v2 design (cost model: matmul time = out_free_size x cyc/row, rhs dtype
keyed; bf16 = 1 cyc/row at any width):
- QKV projection in f32r (full rate at free>=256), biases folded in via
  DVE adds that double as the psum->sbuf drain; q/k land in bf16.
- Scores S[kt,q] in bf16 (64-deep contraction, psum [128, 2 heads, 512]),
  exp fused across 2 heads per ACT instruction (halves ACT instr count).
- p@v computed TRANSPOSED: py[q, d+1] = p[kt,q]^T @ vaug[kt, d|1], so the
  matmul free dim is 65 instead of the 128..512 query width -- pv PE rows
  drop 2x. Denominator rides along as vaug's ones column; the normalize
  becomes a per-partition-scalar broadcast multiply (no partition
  broadcast needed). y is then transposed back with PE transpose-mode
  matmuls (128 rows each) for the output projection.
- Output projection in bf16 from the transposed yT, drained by DVE,
  written f32 to DRAM.
The per-512-token stripes are emitted interleaved; proj for q-tile tg is
emitted one tile behind (lag-1) so its PE work fills the ACT-bound tail.
"""

import os
import sys

for _p in ("/opt/trn_rl_repo", "/opt/pypackages"):
    if os.path.isdir(_p) and _p not in sys.path:
        sys.path.append(_p)

import numpy as np

import concourse.bass as bass
import concourse.tile as tile
import concourse.mybir as mybir
from concourse import bacc
from concourse.bass_utils import run_bass_kernel_spmd

B, T, C = 2, 2048, 1024
H = 16            # total heads
D = 64            # head dim
HPC = 4           # heads per core
CH = HPC * D      # 256 channels per core
N_CORES = 8

f32 = mybir.dt.float32
f32r = mybir.dt.float32r
bf16 = mybir.dt.bfloat16
ts = bass.ts
ds = bass.ds
AF = mybir.ActivationFunctionType

_COMPILED = None


def _build():
    nc = bacc.Bacc("TRN2", target_bir_lowering=False, debug=False,
                   num_devices=N_CORES)

    xT = nc.dram_tensor("xT", [C, T], f32, kind="ExternalInput").ap()
    wt = nc.dram_tensor("wt", [C, 3 * CH], f32, kind="ExternalInput").ap()
    wpt = nc.dram_tensor("wpt", [CH, C], f32, kind="ExternalInput").ap()
    bqk = nc.dram_tensor("bqk", [128, 4], f32, kind="ExternalInput").ap()
    bvb = nc.dram_tensor("bvb", [128, HPC, D], f32, kind="ExternalInput").ap()
    Sm = nc.dram_tensor("Sm", [128, 128], f32, kind="ExternalInput").ap()
    Idm = nc.dram_tensor("Idm", [128, 128], f32, kind="ExternalInput").ap()
    out = nc.dram_tensor("out_partial", [T, C], f32, kind="ExternalOutput").ap()

    NT512 = T // 512          # 4   512-token stripes
    NT128 = T // 128          # 16  128-token tiles
    NC128 = C // 128          # 8   contraction tiles

    with tile.TileContext(nc) as tc:
        with tc.tile_pool(name="consts", bufs=1) as consts, \
             tc.tile_pool(name="qkv", bufs=1) as qkv, \
             tc.tile_pool(name="xp", bufs=2) as xp, \
             tc.tile_pool(name="pp", bufs=17) as pp, \
             tc.tile_pool(name="yn", bufs=2) as yn, \
             tc.tile_pool(name="op", bufs=3) as op, \
             tc.tile_pool(name="ps_s", bufs=2, space="PSUM") as ps_s, \
             tc.tile_pool(name="ps_y", bufs=2, space="PSUM") as ps_y, \
             tc.tile_pool(name="ps_big", bufs=2, space="PSUM") as ps_big:

            # ---- constants; DMA emission order puts stripe-0 essentials
            #      (xt0 chunks + qk half of wt) first ----
            xT_r = xT.rearrange("(o p) t -> p o t", p=128).bitcast(f32r)
            wt_r = wt.rearrange("(o p) f -> p o f", p=128).bitcast(f32r)
            wt_sb = consts.tile([128, NC128, 3 * CH], f32r)
            xt0 = xp.tile([128, NC128, 512], f32r, tag="xt")
            for ci in range(NC128):
                nc.sync.dma_start(xt0[:, ci], xT_r[:, ci, ts(0, 512)])
                nc.sync.dma_start(wt_sb[:, ci, :512], wt_r[:, ci, :512])
            bqk_sb = consts.tile([128, 4], f32)
            nc.sync.dma_start(bqk_sb[:], bqk)
            nc.sync.dma_start(wt_sb[:, :, 512:], wt_r[:, :, 512:])
            bvb_sb = consts.tile([128, HPC, D], f32)
            nc.sync.dma_start(bvb_sb[:], bvb)
            S_f = consts.tile([128, 128], f32)
            nc.sync.dma_start(S_f[:], Sm)
            Id_f = consts.tile([128, 128], f32)
            nc.sync.dma_start(Id_f[:], Idm)
            wpt_f = consts.tile([128, 2, C], f32)
            nc.sync.dma_start(
                wpt_f[:], wpt.rearrange("(s p) o -> p s o", p=128))

            S_sb = consts.tile([128, 128], bf16)
            nc.vector.tensor_copy(S_sb[:], S_f[:])
            Id_sb = consts.tile([128, 128], bf16)
            nc.vector.tensor_copy(Id_sb[:], Id_f[:])
            wpt_sb = consts.tile([128, 2, C], bf16)
            nc.vector.tensor_copy(wpt_sb[:], wpt_f[:])

            # ---- persistent activations ----
            qT = qkv.tile([128, 2, T], bf16)      # [2h*64, slab, t]
            kT = qkv.tile([128, 2, T], bf16)
            vaug = qkv.tile([128, NT128, HPC, D + 1], bf16)  # [kt, ki, h, d|1]
            yT = qkv.tile([128, 2, T], bf16)

            nc.vector.memset(vaug[:, :, :, D:D + 1], 1.0)

            # ---------------- emission helpers ----------------
            from collections import deque

            xt_tiles = {0: xt0}

            def ensure_xt_dma(ti):
                if ti < NT512 and ti not in xt_tiles:
                    xt = xp.tile([128, NC128, 512], f32r, tag="xt")
                    nc.sync.dma_start(xt[:], xT_r[:, :, ts(ti, 512)])
                    xt_tiles[ti] = xt

            def emit_qk_chain(ti, fj):
                xt = xt_tiles[ti]
                ps = ps_big.tile([128, 512], f32, tag="big")
                for ci in range(NC128):
                    nc.tensor.matmul(
                        ps[:], wt_sb[:, ci, ts(fj, 128)], xt[:, ci, :],
                        start=(ci == 0), stop=(ci == NC128 - 1))
                dest = qT if fj < 2 else kT
                nc.vector.tensor_add(
                    out=dest[:, fj % 2, ts(ti, 512)], in0=ps[:],
                    in1=bqk_sb[:, fj:fj + 1].to_broadcast([128, 512]))

            def emit_v_chain(ti, tj):
                xt = xt_tiles[ti]
                pv = ps_big.tile([128, HPC, D], f32, tag="big")
                for ci in range(NC128):
                    nc.tensor.matmul(
                        pv[:, :, :], xt[:, ci, ts(tj, 128)],
                        wt_sb[:, ci, 512:512 + CH],
                        start=(ci == 0), stop=(ci == NC128 - 1))
                nc.vector.tensor_add(
                    out=vaug[:, 4 * ti + tj, :, 0:D],
                    in0=pv[:, :, :], in1=bvb_sb[:])

            p4_all = [None] * NT128  # per global ki

            def emit_scores(qi, ki):
                j = ki - 4 * qi
                q0 = max(0, 128 * j)
                w = 512 - q0
                p4 = pp.tile([128, HPC, 512], bf16, tag="p4")
                for g in range(2):
                    sc = ps_s.tile([128, 2, 512], f32, tag="sc")
                    for hh in range(2):
                        nc.tensor.matmul(
                            sc[:, hh, q0:],
                            kT[ts(hh, D), g, ts(ki, 128)],
                            qT[ts(hh, D), g, ds(512 * qi + q0, w)],
                            start=True, stop=True)
                    nc.scalar.activation(
                        p4[:, ts(g, 2), q0:], sc[:, :, q0:], AF.Exp)
                    if j >= 0:
                        for hh in range(2):
                            nc.vector.tensor_mul(
                                out=p4[:, 2 * g + hh, q0:q0 + 128],
                                in0=p4[:, 2 * g + hh, q0:q0 + 128],
                                in1=S_sb[:])
                p4_all[ki] = p4

            def emit_pv(tg):
                tg_rel = tg % 4
                py4 = ps_y.tile([128, HPC, D + 1], f32, tag="py")
                for h in range(HPC):
                    for ki in range(tg + 1):
                        nc.tensor.matmul(
                            py4[:, h, :],
                            p4_all[ki][:, h, ts(tg_rel, 128)],
                            vaug[:, ki, h, :],
                            start=(ki == 0), stop=(ki == tg))
                rec4 = yn.tile([128, HPC, 1], f32, tag="rec")
                nc.vector.reciprocal(rec4[:], py4[:, :, D:D + 1])
                y_n = yn.tile([128, HPC, D], bf16, tag="yn")
                nc.vector.tensor_mul(
                    out=y_n[:], in0=py4[:, :, 0:D],
                    in1=rec4.to_broadcast([128, HPC, D]))
                y_ns[tg] = y_n

            y_ns = [None] * NT128

            def emit_transpose(tg):
                yTt = ps_y.tile([128, 2, 128], bf16, tag="py")
                for i in range(2):
                    nc.tensor.transpose(
                        yTt[:, i, :], y_ns[tg][:, ts(i, 2), :], Id_sb[:])
                nc.vector.tensor_copy(yT[:, :, ts(tg, 128)], yTt[:])
                proj_q.append(tg)

            def emit_proj(tg):
                for oi in range(2):
                    po = ps_big.tile([128, 512], f32, tag="big")
                    for s in range(2):
                        nc.tensor.matmul(
                            po[:], yT[:, s, ts(tg, 128)],
                            wpt_sb[:, s, ts(oi, 512)],
                            start=(s == 0), stop=(s == 1))
                    ot = op.tile([128, 512], f32, tag="ot")
                    nc.vector.tensor_copy(ot[:], po[:])
                    nc.sync.dma_start(out[ts(tg, 128), ts(oi, 512)], ot[:])

            # -------- software-pipelined emission --------
            # pv lags the scores by one k-block; transpose lags pv by one
            # tile; proj tiles are held in a queue and spent as PE filler
            # inside ACT-bound score regions (and the tail).
            filler = deque()   # pending PE-heavy units (qkv chains, late v)
            proj_q = deque()   # proj tiles ready to emit
            state = {"pv": 0}

            def advance_pipeline(upto):
                while state["pv"] <= min(upto, NT128 - 1):
                    tg = state["pv"]
                    emit_pv(tg)
                    if tg >= 1:
                        emit_transpose(tg - 1)
                    state["pv"] += 1

            for ti in range(NT512):
                ensure_xt_dma(ti)
                ensure_xt_dma(ti + 1)
                if ti == 0:
                    for fj in range(4):
                        emit_qk_chain(0, fj)
                # v chains: stripes 0-2 inline; stripe 3 deferred as filler
                if ti < 3:
                    for tj in range(4):
                        emit_v_chain(ti, tj)
                else:
                    for tj in range(4):
                        filler.append((emit_v_chain, (3, tj)))
                # next stripe's q/k chains become filler inside our scores
                if ti + 1 < NT512:
                    for fj in range(4):
                        filler.append((emit_qk_chain, (ti + 1, fj)))
                # finish previous stripe's last tiles (their exps are done)
                advance_pipeline(4 * ti - 1)

                nk = 4 * ti + 4
                for ki in range(nk):
                    emit_scores(ti, ki)
                    if ki - 1 >= 4 * ti:
                        advance_pipeline(ki - 1)
                    elif filler:
                        fn, args = filler.popleft()
                        fn(*args)
                    elif proj_q:
                        emit_proj(proj_q.popleft())
                # flush leftover qkv filler before the next stripe needs it
                while filler:
                    fn, args = filler.popleft()
                    fn(*args)

            advance_pipeline(NT128 - 1)
            emit_transpose(NT128 - 1)
            while proj_q:
                emit_proj(proj_q.popleft())

    nc.compile()
    return nc


def _get_compiled():
    global _COMPILED
    if _COMPILED is None:
        _COMPILED = _build()
    return _COMPILED


def _host_prep(x, W_attn, b_attn, W_proj, b_proj):
    scale = 1.0 / np.sqrt(np.float32(D))
    xTb = [np.ascontiguousarray(x[b].T).astype(np.float32) for b in range(B)]
    Sm = (np.arange(128, dtype=np.int32)[None, :]
          >= np.arange(128, dtype=np.int32)[:, None]).astype(np.float32)
    Idm = np.eye(128, dtype=np.float32)
    in_maps = []
    for c in range(N_CORES):
        b, g = divmod(c, 4)
        ch = slice(CH * g, CH * (g + 1))
        Wq = W_attn[ch]
        Wk = W_attn[C:][ch] * scale
        Wv = W_attn[2 * C:][ch]
        wt_c = np.ascontiguousarray(
            np.concatenate([Wq, Wk, Wv], axis=0).T).astype(np.float32)
        bq = b_attn[ch]
        bk = b_attn[C:][ch] * scale
        bv = b_attn[2 * C:][ch]
        bqk_c = np.ascontiguousarray(
            np.concatenate([bq, bk]).reshape(4, 128).T).astype(np.float32)
        bvb_c = np.ascontiguousarray(
            np.broadcast_to(bv[None, :].reshape(1, HPC, D),
                            (128, HPC, D))).astype(np.float32)
        wpt_c = np.ascontiguousarray(W_proj[:, ch].T).astype(np.float32)
        in_maps.append({
            "xT": xTb[b],
            "wt": wt_c,
            "wpt": wpt_c,
            "bqk": bqk_c,
            "bvb": bvb_c,
            "Sm": Sm,
            "Idm": Idm,
        })
    return in_maps


def kernel(x, W_attn, b_attn, W_proj, b_proj):
    x = np.asarray(x, dtype=np.float32)
    W_attn = np.asarray(W_attn, dtype=np.float32)
    b_attn = np.asarray(b_attn, dtype=np.float32)
    W_proj = np.asarray(W_proj, dtype=np.float32)
    b_proj = np.asarray(b_proj, dtype=np.float32)

    nc = _get_compiled()
    in_maps = _host_prep(x, W_attn, b_attn, W_proj, b_proj)
    res = run_bass_kernel_spmd(nc, in_maps, core_ids=list(range(N_CORES)))

    out = np.empty((B, T, C), dtype=np.float32)
    for b in range(B):
        acc = res.results[4 * b]["out_partial"].copy()
        for g in range(1, 4):
            acc += res.results[4 * b + g]["out_partial"]
        out[b] = acc + b_proj
    return out
